# revision 35
# baseline (speedup 1.0000x reference)
"""EMA dechunker kernel for Trainium2 (Bass/Tile), 8-core data-parallel.

Problem: for each batch row
  smoothed[j] = m[j] ? clip(p[j])*emb[j] + (1-clip(p[j]))*smoothed[j-1]
                     : smoothed[j-1]
  frames[l]   = smoothed[clip(cumsum(boundary)[l]-1, 0, J-1)]

Sharding: batch dim B=16 split across 8 cores (2 rows/core).

v5 design:
 - EMA as blocked matmuls in NATURAL layout (no PE transposes of the
   data, no DVE scans): within each 128-step chunk, s = L @ x + d*carry
   where L[i,j] = prod_{j<k<=i} a_k = exp(cs[i]-cs[j]) (lower-tri),
   cs = per-chunk cumsum(log a) via a tri128 matmul. L is built with one
   DVE add (cs-row broadcast from PSUM + additive -1e9 tri mask) and one
   fused ACT exp (bias = -cs column). The carry enters as a second
   matmul whose lhsT is nonzero only in partition 127 (decay row,
   DMA-filled), reading the previous chunk's bf16 smoothed tile.
 - Coefficients are computed directly in chunk-per-partition [16, 128]
   stage layout (one DMA from DRAM, short dependency chain).
 - smoothed is bf16 in DRAM (halves gather traffic; tol is 2e-2, bf16
   costs ~0.3%; f32 PSUM keeps the recurrence accurate).
 - Upsample gather via SWDGE dma_gather in a SCATTER-WRAP index order:
   descriptor i of sub s fetches frame l = (i%16)*256 + 64 s + (i//16),
   so adjacent descriptors hit distant smoothed rows and the gather
   ucode cannot coalesce them into its slow 64-KiB broadcast
   descriptors (measured 5-11 GB/s/engine vs 21-22 for 1-KiB ones).
   The output store's DRAM access pattern inverts the permutation.
 - Output stores cast bf16->f32 inside the DMA on even subs (gpsimd
   DMAs can cast); odd subs convert on DVE/ACT + store via SP HWDGE,
   spreading queue load.
"""

from contextlib import ExitStack

import numpy as np

import concourse.bass as bass
import concourse.tile as tile
from concourse import bacc, mybir
from concourse.bass_utils import run_bass_kernel_spmd
from concourse.masks import make_identity

F32 = mybir.dt.float32
BF16 = mybir.dt.bfloat16
I16 = mybir.dt.int16
U8 = mybir.dt.uint8
OP = mybir.AluOpType
AF = mybir.ActivationFunctionType

B, J, L, D = 16, 1024, 4096, 512
N_CORES = 8
BL = B // N_CORES          # 2 batch rows per core
T = 128                    # j-chunk size (PSUM block)
NCH = J // T               # 8 chunks per row
NC2 = BL * NCH             # 16 chunk columns (r*NCH + c)
NSUB = 4                   # sub-gathers per row
SUBL = L // NSUB           # 1024 frames per sub-gather
GSUB = SUBL // T           # 8 gather groups of 128 frames
EPS = 1e-4


def _body(tc, ctx):
    nc = tc.nc
    emb = nc.dram_tensor("unit_embeddings", [BL, J, D], F32, kind="ExternalInput").ap()
    conf = nc.dram_tensor("unit_confidence", [BL, J], F32, kind="ExternalInput").ap()
    mask = nc.dram_tensor("unit_mask", [BL, J], U8, kind="ExternalInput").ap()
    bdry = nc.dram_tensor("boundary_mask", [BL, L], U8, kind="ExternalInput").ap()
    out = nc.dram_tensor("frames", [BL, L, D], F32, kind="ExternalOutput").ap()
    smoothed = [
        nc.dram_tensor(f"smoothed{r}", [J, D], BF16, kind="Internal").ap()
        for r in range(BL)
    ]

    const_p = ctx.enter_context(tc.tile_pool(name="const", bufs=1))
    coef_p = ctx.enter_context(tc.tile_pool(name="coef", bufs=1))
    e_p = ctx.enter_context(tc.tile_pool(name="e", bufs=16))
    x_p = ctx.enter_context(tc.tile_pool(name="x", bufs=4))
    lt_p = ctx.enter_context(tc.tile_pool(name="lt", bufs=4))
    smn_p = ctx.enter_context(tc.tile_pool(name="smn", bufs=3))
    idx_p = ctx.enter_context(tc.tile_pool(name="idx", bufs=1))
    gout_p = ctx.enter_context(tc.tile_pool(name="gout", bufs=4))
    stg_p = ctx.enter_context(tc.tile_pool(name="stg", bufs=2))
    psum_p = ctx.enter_context(tc.tile_pool(name="psum", bufs=2, space="PSUM"))
    psc_p = ctx.enter_context(tc.tile_pool(name="psc", bufs=3, space="PSUM"))

    ps_ctr = [0]

    def ps_tile(shape, dtype=F32):
        ps_ctr[0] += 1
        return psum_p.tile(shape, dtype, tag="ps", name=f"ps{ps_ctr[0]}")

    # --- constants ---
    ident = const_p.tile([128, 128], F32)
    make_identity(nc, ident[:])
    zeros128 = const_p.tile([128, 128], F32)
    nc.gpsimd.memset(zeros128[:], 0.0)
    # tri128[k, p] = 1 iff k <= p (inclusive-cumsum lhsT)
    tri128 = const_p.tile([128, 128], F32)
    nc.vector.tensor_tensor_scan(
        out=tri128[:], data0=zeros128[:], data1=ident[:],
        initial=0.0, op0=OP.add, op1=OP.add,
    )
    # additive mask: 0 where k <= p (keep), -1e9 where k > p
    trimask = const_p.tile([128, 128], F32)
    nc.vector.tensor_scalar(
        out=trimask[:], in0=tri128[:], scalar1=1e9, scalar2=-1e9,
        op0=OP.mult, op1=OP.add,
    )
    ones_row = const_p.tile([1, 128], F32)
    nc.gpsimd.memset(ones_row[:], 1.0)
    # strict lower tri16 (k < p) for cross-partition exclusive prefix
    tri16x = const_p.tile([16, 16], F32)
    nc.vector.tensor_tensor_scan(
        out=tri16x[:], data0=zeros128[:16, :16], data1=ident[:16, :16],
        initial=0.0, op0=OP.add, op1=OP.add,
    )
    nc.vector.tensor_tensor(
        out=tri16x[:], in0=tri16x[:], in1=ident[:16, :16], op=OP.subtract
    )
    zerocol16 = const_p.tile([16, 256], F32)
    nc.gpsimd.memset(zerocol16[:], 0.0)

    # --- all embedding loads enqueued first (no deps; SP queue) ---
    e_tiles = {}
    for r in range(BL):
        for c in range(NCH):
            e_c = e_p.tile([T, D], F32, tag="e", name=f"e{r}_{c}")
            nc.sync.dma_start(e_c[:], emb[r, c * T : (c + 1) * T, :])
            e_tiles[(r, c)] = e_c

    # dummy gather to preload the SWDGE gather ucode (~8 us first-use cost)
    dummy_dram = nc.dram_tensor("gdummy", [1, 128], BF16, kind="Internal").ap()
    zidx = const_p.tile([128, 8], I16)
    nc.gpsimd.memset(zidx[:], 0)
    gdt = const_p.tile([128, 1, 128], BF16)
    nc.gpsimd.dma_gather(
        out_ap=gdt[:], in_ap=dummy_dram[:], idxs_ap=zidx[:],
        num_idxs=128, num_idxs_reg=128, elem_size=128, queue_num=1,
    )

    # --- coefficients in chunk-per-partition stage layout [NC2, 128] ---
    cfstage = coef_p.tile([NC2, T], F32)
    mstage = coef_p.tile([NC2, T], F32)
    for r in range(BL):
        nc.scalar.dma_start(
            cfstage[r * NCH : (r + 1) * NCH, :],
            conf[r : r + 1, :].rearrange("one (c t) -> (one c) t", c=NCH),
        )
        nc.gpsimd.dma_start(
            mstage[r * NCH : (r + 1) * NCH, :],
            mask[r : r + 1, :].rearrange("one (c t) -> (one c) t", c=NCH),
        )  # u8 -> f32 cast
    cstage = coef_p.tile([NC2, T], F32)
    nc.vector.tensor_scalar(
        out=cstage[:], in0=cfstage[:], scalar1=EPS, scalar2=1.0 - EPS,
        op0=OP.max, op1=OP.min,
    )
    nc.vector.tensor_tensor(out=cstage[:], in0=cstage[:], in1=mstage[:], op=OP.mult)
    astage = coef_p.tile([NC2, T], F32)
    nc.vector.tensor_scalar(
        out=astage[:], in0=cstage[:], scalar1=-1.0, scalar2=1.0,
        op0=OP.mult, op1=OP.add,
    )
    lstage = coef_p.tile([NC2, T], F32)
    nc.scalar.activation(lstage[:], astage[:], AF.Ln)

    # wrap to step-per-partition [128, NC2] via PE transpose
    c_wrap = coef_p.tile([128, NC2], F32)
    ps_cw = ps_tile([128, NC2])
    nc.tensor.matmul(
        out=ps_cw[:], lhsT=cstage[:], rhs=ident[:NC2, :NC2], start=True, stop=True,
        is_transpose=True,
    )
    nc.vector.tensor_copy(c_wrap[:], ps_cw[:])
    la_wrap = coef_p.tile([128, NC2], F32)
    ps_lw = ps_tile([128, NC2])
    nc.tensor.matmul(
        out=ps_lw[:], lhsT=lstage[:], rhs=ident[:NC2, :NC2], start=True, stop=True,
        is_transpose=True,
    )
    nc.scalar.copy(la_wrap[:], ps_lw[:])

    # cs = per-chunk inclusive cumsum of log(a): tri128 @ la_wrap
    ps_cs = ps_tile([128, NC2])
    nc.tensor.matmul(out=ps_cs[:], lhsT=tri128[:], rhs=la_wrap[:], start=True, stop=True)
    negcs_col = coef_p.tile([128, NC2], F32)
    nc.vector.tensor_scalar_mul(negcs_col[:], ps_cs[:], -1.0)
    cs_col = coef_p.tile([128, NC2], F32)
    nc.scalar.copy(cs_col[:], ps_cs[:])
    # chunk-major flat cs on partition 0: [1, col*128 + i] = cs[i, col]
    ps_csT = ps_tile([NC2, 128])
    nc.tensor.matmul(
        out=ps_csT[:], lhsT=cs_col[:], rhs=ident[:], start=True, stop=True,
        is_transpose=True,
    )
    csT = coef_p.tile([NC2, 128], F32)
    nc.vector.tensor_copy(csT[:], ps_csT[:])
    cs_row0 = coef_p.tile([1, NC2 * 128], F32)
    nc.scalar.dma_start(cs_row0[:], csT[:])
    # decay rows d[i] = exp(cs[i]) (bf16), contiguous per chunk
    d_all = coef_p.tile([1, NC2 * 128], BF16)
    nc.scalar.activation(d_all[:], cs_row0[:], AF.Exp)

    # carry-injection lhsTs: nonzero only in partition 127 (decay row).
    # Engines can't address partition 127 alone; DMAs can.
    carry_all = []
    for r in range(BL):
        ca = const_p.tile([128, NCH * 128], BF16, tag=f"clhs{r}", name=f"clhs{r}")
        nc.gpsimd.memset(ca[:], 0.0)
        for c in range(1, NCH):
            col = r * NCH + c
            nc.sync.dma_start(
                ca[127:128, c * 128 : (c + 1) * 128],
                d_all[:, col * 128 : (col + 1) * 128],
            )
        carry_all.append(ca)

    def ema_row(r):
        smn_prev = None
        for c in range(NCH):
            col = r * NCH + c
            e_c = e_tiles[(r, c)]
            # x = c * emb (bf16)
            x_c = x_p.tile([T, D], BF16, tag="x", name=f"x{r}_{c}")
            nc.vector.tensor_tensor(
                out=x_c[:], in0=e_c[:],
                in1=c_wrap[:, col : col + 1].to_broadcast([T, D]), op=OP.mult,
            )
            # L^T[j, i] = exp(cs[i] - cs[j]) masked to j <= i
            ps_rbc = ps_tile([128, 128])
            nc.tensor.matmul(
                out=ps_rbc[:], lhsT=ones_row[:],
                rhs=cs_row0[:, col * 128 : (col + 1) * 128],
                start=True, stop=True,
            )
            rowm = lt_p.tile([128, 128], F32, tag="rowm", name=f"rowm{r}_{c}")
            nc.vector.tensor_tensor(
                out=rowm[:], in0=ps_rbc[:], in1=trimask[:], op=OP.add,
            )
            lt = lt_p.tile([128, 128], BF16, tag="lt", name=f"lt{r}_{c}")
            nc.scalar.activation(
                lt[:], rowm[:], AF.Exp, bias=negcs_col[:, col : col + 1]
            )
            ps_c = psc_p.tile([T, D], F32, tag="psc", name=f"psc{r}_{c}")
            if c == 0:
                nc.tensor.matmul(
                    out=ps_c[:], lhsT=lt[:], rhs=x_c[:], start=True, stop=True
                )
            else:
                nc.tensor.matmul(
                    out=ps_c[:], lhsT=lt[:], rhs=x_c[:], start=True, stop=False
                )
                nc.tensor.matmul(
                    out=ps_c[:], lhsT=carry_all[r][:, c * 128 : (c + 1) * 128],
                    rhs=smn_prev[:], start=False, stop=True,
                )
            smn = smn_p.tile([T, D], BF16, tag="smn", name=f"smn{r}_{c}")
            if c % 2 == 0:
                nc.vector.tensor_copy(smn[:], ps_c[:])
                nc.sync.dma_start(smoothed[r][c * T : (c + 1) * T, :], smn[:])
            else:
                nc.scalar.copy(smn[:], ps_c[:])
                nc.scalar.dma_start(smoothed[r][c * T : (c + 1) * T, :], smn[:])
            smn_prev = smn

    # --- indices, scatter-wrap: idx16[q, t] = clip(cumsum-1)[q*256 + t] ---
    idx_rep = []

    def idx_row(r):
        w16 = idx_p.tile([16, 256], F32, tag=f"w{r}")
        nc.gpsimd.dma_start(
            w16[:], bdry[r : r + 1, :].rearrange("one (p q) -> (one p) q", p=16)
        )  # u8 -> f32 cast
        incl = idx_p.tile([16, 256], F32, tag=f"incl{r}")
        nc.vector.tensor_tensor_scan(
            out=incl[:], data0=w16[:], data1=zerocol16[:],
            initial=0.0, op0=OP.add, op1=OP.add,
        )
        # cross-partition exclusive prefix of per-partition totals
        ps_off = ps_tile([16, 1])
        nc.tensor.matmul(
            out=ps_off[:], lhsT=tri16x[:], rhs=incl[:, 255:256], start=True, stop=True
        )
        total = idx_p.tile([16, 256], F32, tag=f"tot{r}")
        nc.vector.tensor_tensor(
            out=total[:], in0=incl[:], in1=ps_off[:].to_broadcast([16, 256]), op=OP.add
        )
        idxf = idx_p.tile([16, 256], F32, tag=f"idxf{r}")
        nc.vector.tensor_scalar(
            out=idxf[:], in0=total[:], scalar1=-1.0, scalar2=0.0, op0=OP.add, op1=OP.max
        )
        nc.vector.tensor_scalar_min(idxf[:], idxf[:], float(J - 1))
        idx16 = idx_p.tile([16, 256], I16, tag=f"idx16{r}")
        nc.vector.tensor_copy(idx16[:], idxf[:])
        # idx16[q, t] = idx[q*256 + t] IS the scatter-wrap fed order:
        # descriptor i of sub s fetches frame (i%16)*256 + 64 s + i//16.
        # replicate to 128 partitions (8 gpsimd core groups) by doubling
        rep = idx_p.tile([128, 256], I16, tag=f"rep{r}")
        nc.sync.dma_start(rep[0:16, :], idx16[:])
        nc.sync.dma_start(rep[16:32, :], rep[0:16, :])
        nc.sync.dma_start(rep[32:64, :], rep[0:32, :])
        nc.sync.dma_start(rep[64:128, :], rep[0:64, :])
        idx_rep.append(rep)

    def gather_sub(r, s):
        gt = gout_p.tile([128, GSUB, D], BF16, tag="gout", name=f"gout{r}_{s}")
        nc.gpsimd.dma_gather(
            out_ap=gt[:],
            in_ap=smoothed[r][:],
            idxs_ap=idx_rep[r][:, s * (SUBL // 16) : (s + 1) * (SUBL // 16)],
            num_idxs=SUBL,
            num_idxs_reg=SUBL,
            elem_size=D,
            queue_num=(r * NSUB + s) % 3 + 1,  # queue 0 is the mainline ring
        )
        return gt

    def store_sub(r, s, gt):
        # gt[p, g] holds frame l = (p%16)*256 + 64 s + 8 g + p//16. The
        # inverse view needs (ph, pl, g, d) — 4 dims exceed the DMA AP
        # balance limit, so store in 8 per-ph pieces of [16, 8, D].
        dst4 = out[r, :, :].rearrange(
            "(pl ss g ph) d -> ss ph pl g d", pl=16, ss=NSUB, g=GSUB
        )[s]
        if s % 2 == 0:
            for ph in range(8):
                nc.gpsimd.dma_start(dst4[ph], gt[ph * 16 : (ph + 1) * 16, :, :])
        else:
            stg = stg_p.tile([128, GSUB, D], F32, tag="stg", name=f"stg{r}_{s}")
            if (r + s) % 4 < 2:
                nc.vector.tensor_copy(stg[:], gt[:])
            else:
                nc.scalar.copy(stg[:], gt[:])
            for ph in range(8):
                eng = nc.sync if ph % 2 == 0 else nc.scalar
                eng.dma_start(dst4[ph], stg[ph * 16 : (ph + 1) * 16, :, :])

    ema_row(0)
    idx_row(0)
    gts0 = [gather_sub(0, s) for s in range(NSUB)]
    idx_row(1)
    ema_row(1)
    gts1 = []
    for s in range(NSUB):
        store_sub(0, s, gts0[s])
        gts1.append(gather_sub(1, s))
    for s in range(NSUB):
        store_sub(1, s, gts1[s])


def _patch_swdge_lane_by_queue():
    """Tile assigns DMASW completion-sem lanes round-robin, queue-blind; the
    HW/sim lock each lane's sem to one SWDGE queue. Pin lane = queue_num so
    multi-queue pool DMAs get consistent lanes."""
    from concourse import bass_isa
    from concourse import tile_sem_assignment as tsa

    if getattr(tsa.TileClockTick, "_ema_queue_patch", False):
        return
    orig = tsa.TileClockTick._assign_tick

    def patched(self, inst):
        if (
            isinstance(inst, bass_isa.AnyDMAInstruction)
            and inst.engine == mybir.EngineType.Pool
            and not isinstance(inst, bass_isa.UserSyncedRemoteDMADescs)
        ):
            self.next_sw_dma_idx = getattr(inst, "queue_num", 0) or 0
        return orig(self, inst)

    tsa.TileClockTick._assign_tick = patched
    tsa.TileClockTick._ema_queue_patch = True


def build():
    _patch_swdge_lane_by_queue()
    nc = bacc.Bacc(
        "TRN2",
        target_bir_lowering=False,
        debug=False,
        enable_asserts=False,
        num_devices=N_CORES,
        num_swdge_queues=4,
        dynamic_dma_scratch_size=16384,
    )
    with tile.TileContext(nc) as tc, ExitStack() as ctx:
        _body(tc, ctx)
    nc.compile()
    return nc


def make_in_maps(inputs):
    emb = np.asarray(inputs["unit_embeddings"], dtype=np.float32)
    conf = np.asarray(inputs["unit_confidence"], dtype=np.float32)
    msk = np.asarray(inputs["unit_mask"]).astype(np.uint8)
    bd = np.asarray(inputs["boundary_mask"]).astype(np.uint8)
    in_maps = []
    for c in range(N_CORES):
        sl = slice(c * BL, (c + 1) * BL)
        in_maps.append(
            {
                "unit_embeddings": np.ascontiguousarray(emb[sl]),
                "unit_confidence": np.ascontiguousarray(conf[sl]),
                "unit_mask": np.ascontiguousarray(msk[sl]),
                "boundary_mask": np.ascontiguousarray(bd[sl]),
            }
        )
    return in_maps


_cached_nc = None


def run(inputs, trace=False):
    global _cached_nc
    if _cached_nc is None:
        _cached_nc = build()
    res = run_bass_kernel_spmd(
        _cached_nc, make_in_maps(inputs), core_ids=list(range(N_CORES)), trace=trace
    )
    full = np.concatenate(
        [res.results[c]["frames"] for c in range(N_CORES)], axis=0
    )
    return full, res


def kernel(**inputs) -> np.ndarray:
    import os

    # Trace capture needs hooks absent outside our dev harness; make sure a
    # stray BASS_TRACE env can't route the grading run down that path.
    prev = os.environ.get("BASS_NEVER_TRACE")
    os.environ["BASS_NEVER_TRACE"] = "1"
    try:
        full, _ = run(inputs, trace=False)
    finally:
        if prev is None:
            os.environ.pop("BASS_NEVER_TRACE", None)
        else:
            os.environ["BASS_NEVER_TRACE"] = prev
    return full


# revision 39
# speedup vs baseline: 1.3403x; 1.3403x over previous
"""EMA dechunker kernel for Trainium2 (Bass/Tile), 8-core data-parallel.

Problem: for each batch row
  smoothed[j] = m[j] ? clip(p[j])*emb[j] + (1-clip(p[j]))*smoothed[j-1]
                     : smoothed[j-1]
  frames[l]   = smoothed[clip(cumsum(boundary)[l]-1, 0, J-1)]

Sharding: batch dim B=16 split across 8 cores (2 rows/core).

v5 design:
 - EMA as blocked matmuls in NATURAL layout (no PE transposes of the
   data, no DVE scans): within each 128-step chunk, s = L @ x + d*carry
   where L[i,j] = prod_{j<k<=i} a_k = exp(cs[i]-cs[j]) (lower-tri),
   cs = per-chunk cumsum(log a) via a tri128 matmul. L is built with one
   DVE add (cs-row broadcast from PSUM + additive -1e9 tri mask) and one
   fused ACT exp (bias = -cs column). The carry enters as a second
   matmul whose lhsT is nonzero only in partition 127 (decay row,
   DMA-filled), reading the previous chunk's bf16 smoothed tile.
 - Coefficients are computed directly in chunk-per-partition [16, 128]
   stage layout (one DMA from DRAM, short dependency chain).
 - smoothed is bf16 in DRAM (halves gather traffic; tol is 2e-2, bf16
   costs ~0.3%; f32 PSUM keeps the recurrence accurate).
 - Upsample gather via SWDGE dma_gather in a SCATTER-WRAP index order:
   descriptor i of sub s fetches frame l = (i%16)*256 + 64 s + (i//16),
   so adjacent descriptors hit distant smoothed rows and the gather
   ucode cannot coalesce them into its slow 64-KiB broadcast
   descriptors (measured 5-11 GB/s/engine vs 21-22 for 1-KiB ones).
   The output store's DRAM access pattern inverts the permutation.
 - Output stores cast bf16->f32 inside the DMA on even subs (gpsimd
   DMAs can cast); odd subs convert on DVE/ACT + store via SP HWDGE,
   spreading queue load.
"""

from contextlib import ExitStack

import numpy as np

import concourse.bass as bass
import concourse.tile as tile
from concourse import bacc, mybir
from concourse.bass_utils import run_bass_kernel_spmd
from concourse.masks import make_identity

F32 = mybir.dt.float32
BF16 = mybir.dt.bfloat16
I16 = mybir.dt.int16
U8 = mybir.dt.uint8
OP = mybir.AluOpType
AF = mybir.ActivationFunctionType

B, J, L, D = 16, 1024, 4096, 512
N_CORES = 8
BL = B // N_CORES          # 2 batch rows per core
T = 128                    # j-chunk size (PSUM block)
NCH = J // T               # 8 chunks per row
NC2 = BL * NCH             # 16 chunk columns (r*NCH + c)
NSUB = 4                   # sub-gathers per row
SUBL = L // NSUB           # 1024 frames per sub-gather
GSUB = SUBL // T           # 8 gather groups of 128 frames
EPS = 1e-4


def _body(tc, ctx):
    nc = tc.nc
    emb = nc.dram_tensor("unit_embeddings", [BL, J, D], F32, kind="ExternalInput").ap()
    conf = nc.dram_tensor("unit_confidence", [BL, J], F32, kind="ExternalInput").ap()
    mask = nc.dram_tensor("unit_mask", [BL, J], U8, kind="ExternalInput").ap()
    bdry = nc.dram_tensor("boundary_mask", [BL, L], U8, kind="ExternalInput").ap()
    out = nc.dram_tensor("frames", [BL, L, D], F32, kind="ExternalOutput").ap()
    smoothed = [
        nc.dram_tensor(f"smoothed{r}", [J, D], BF16, kind="Internal").ap()
        for r in range(BL)
    ]

    const_p = ctx.enter_context(tc.tile_pool(name="const", bufs=1))
    coef_p = ctx.enter_context(tc.tile_pool(name="coef", bufs=1))
    e_p = ctx.enter_context(tc.tile_pool(name="e", bufs=16))
    x_p = ctx.enter_context(tc.tile_pool(name="x", bufs=4))
    lt_p = ctx.enter_context(tc.tile_pool(name="lt", bufs=4))
    smn_p = ctx.enter_context(tc.tile_pool(name="smn", bufs=3))
    idx_p = ctx.enter_context(tc.tile_pool(name="idx", bufs=1))
    gout_p = ctx.enter_context(tc.tile_pool(name="gout", bufs=4))
    stg_p = ctx.enter_context(tc.tile_pool(name="stg", bufs=2))
    psum_p = ctx.enter_context(tc.tile_pool(name="psum", bufs=2, space="PSUM"))
    psc_p = ctx.enter_context(tc.tile_pool(name="psc", bufs=3, space="PSUM"))

    ps_ctr = [0]

    def ps_tile(shape, dtype=F32):
        ps_ctr[0] += 1
        return psum_p.tile(shape, dtype, tag="ps", name=f"ps{ps_ctr[0]}")

    # --- constants ---
    ident = const_p.tile([128, 128], F32)
    make_identity(nc, ident[:])
    zeros128 = const_p.tile([128, 128], F32)
    nc.gpsimd.memset(zeros128[:], 0.0)
    # tri128[k, p] = 1 iff k <= p (inclusive-cumsum lhsT)
    tri128 = const_p.tile([128, 128], F32)
    nc.vector.tensor_tensor_scan(
        out=tri128[:], data0=zeros128[:], data1=ident[:],
        initial=0.0, op0=OP.add, op1=OP.add,
    )
    # additive mask: 0 where k <= p (keep), -1e9 where k > p
    trimask = const_p.tile([128, 128], F32)
    nc.vector.tensor_scalar(
        out=trimask[:], in0=tri128[:], scalar1=1e9, scalar2=-1e9,
        op0=OP.mult, op1=OP.add,
    )
    ones_row = const_p.tile([1, 128], F32)
    nc.gpsimd.memset(ones_row[:], 1.0)
    # tri16[k, p] = 1 iff k <= p (partition-dim inclusive cumsum)
    tri16 = const_p.tile([16, 16], F32)
    nc.vector.tensor_tensor_scan(
        out=tri16[:], data0=zeros128[:16, :16], data1=ident[:16, :16],
        initial=0.0, op0=OP.add, op1=OP.add,
    )
    ones_col16 = const_p.tile([16, 1], F32)
    nc.gpsimd.memset(ones_col16[:], 1.0)
    zeros_row256 = const_p.tile([1, 256], F32)
    nc.gpsimd.memset(zeros_row256[:], 0.0)

    # --- all embedding loads enqueued first (no deps; SP queue) ---
    e_tiles = {}
    for r in range(BL):
        for c in range(NCH):
            e_c = e_p.tile([T, D], F32, tag="e", name=f"e{r}_{c}")
            nc.sync.dma_start(e_c[:], emb[r, c * T : (c + 1) * T, :])
            e_tiles[(r, c)] = e_c

    # dummy gather to preload the SWDGE gather ucode (~8 us first-use cost)
    dummy_dram = nc.dram_tensor("gdummy", [1, 128], BF16, kind="Internal").ap()
    zidx = const_p.tile([128, 8], I16)
    nc.gpsimd.memset(zidx[:], 0)
    gdt = const_p.tile([128, 1, 128], BF16)
    nc.gpsimd.dma_gather(
        out_ap=gdt[:], in_ap=dummy_dram[:], idxs_ap=zidx[:],
        num_idxs=128, num_idxs_reg=128, elem_size=128, queue_num=1,
    )

    # --- coefficients in chunk-per-partition stage layout [NC2, 128] ---
    cfstage = coef_p.tile([NC2, T], F32)
    mstage = coef_p.tile([NC2, T], F32)
    for r in range(BL):
        nc.scalar.dma_start(
            cfstage[r * NCH : (r + 1) * NCH, :],
            conf[r : r + 1, :].rearrange("one (c t) -> (one c) t", c=NCH),
        )
        nc.gpsimd.dma_start(
            mstage[r * NCH : (r + 1) * NCH, :],
            mask[r : r + 1, :].rearrange("one (c t) -> (one c) t", c=NCH),
        )  # u8 -> f32 cast
    cstage = coef_p.tile([NC2, T], F32)
    nc.vector.tensor_scalar(
        out=cstage[:], in0=cfstage[:], scalar1=EPS, scalar2=1.0 - EPS,
        op0=OP.max, op1=OP.min,
    )
    nc.vector.tensor_tensor(out=cstage[:], in0=cstage[:], in1=mstage[:], op=OP.mult)
    astage = coef_p.tile([NC2, T], F32)
    nc.vector.tensor_scalar(
        out=astage[:], in0=cstage[:], scalar1=-1.0, scalar2=1.0,
        op0=OP.mult, op1=OP.add,
    )
    lstage = coef_p.tile([NC2, T], F32)
    nc.scalar.activation(lstage[:], astage[:], AF.Ln)

    # wrap to step-per-partition [128, NC2] via PE transpose
    c_wrap = coef_p.tile([128, NC2], F32)
    ps_cw = ps_tile([128, NC2])
    nc.tensor.matmul(
        out=ps_cw[:], lhsT=cstage[:], rhs=ident[:NC2, :NC2], start=True, stop=True,
        is_transpose=True,
    )
    nc.vector.tensor_copy(c_wrap[:], ps_cw[:])
    la_wrap = coef_p.tile([128, NC2], F32)
    ps_lw = ps_tile([128, NC2])
    nc.tensor.matmul(
        out=ps_lw[:], lhsT=lstage[:], rhs=ident[:NC2, :NC2], start=True, stop=True,
        is_transpose=True,
    )
    nc.scalar.copy(la_wrap[:], ps_lw[:])

    # cs = per-chunk inclusive cumsum of log(a): tri128 @ la_wrap
    ps_cs = ps_tile([128, NC2])
    nc.tensor.matmul(out=ps_cs[:], lhsT=tri128[:], rhs=la_wrap[:], start=True, stop=True)
    negcs_col = coef_p.tile([128, NC2], F32)
    nc.vector.tensor_scalar_mul(negcs_col[:], ps_cs[:], -1.0)
    cs_col = coef_p.tile([128, NC2], F32)
    nc.scalar.copy(cs_col[:], ps_cs[:])
    # chunk-major flat cs on partition 0: [1, col*128 + i] = cs[i, col]
    ps_csT = ps_tile([NC2, 128])
    nc.tensor.matmul(
        out=ps_csT[:], lhsT=cs_col[:], rhs=ident[:], start=True, stop=True,
        is_transpose=True,
    )
    csT = coef_p.tile([NC2, 128], F32)
    nc.vector.tensor_copy(csT[:], ps_csT[:])
    cs_row0 = coef_p.tile([1, NC2 * 128], F32)
    nc.scalar.dma_start(cs_row0[:], csT[:])
    # decay rows d[i] = exp(cs[i]) (bf16), contiguous per chunk
    d_all = coef_p.tile([1, NC2 * 128], BF16)
    nc.scalar.activation(d_all[:], cs_row0[:], AF.Exp)

    # carry-injection lhsTs: nonzero only in partition 127 (decay row).
    # Engines can't address partition 127 alone; DMAs can.
    carry_all = []
    for r in range(BL):
        ca = const_p.tile([128, NCH * 128], BF16, tag=f"clhs{r}", name=f"clhs{r}")
        nc.gpsimd.memset(ca[:], 0.0)
        for c in range(1, NCH):
            col = r * NCH + c
            nc.sync.dma_start(
                ca[127:128, c * 128 : (c + 1) * 128],
                d_all[:, col * 128 : (col + 1) * 128],
            )
        carry_all.append(ca)

    def ema_row(r):
        smn_prev = None
        for c in range(NCH):
            col = r * NCH + c
            e_c = e_tiles[(r, c)]
            # x = c * emb (bf16)
            x_c = x_p.tile([T, D], BF16, tag="x", name=f"x{r}_{c}")
            nc.vector.tensor_tensor(
                out=x_c[:], in0=e_c[:],
                in1=c_wrap[:, col : col + 1].to_broadcast([T, D]), op=OP.mult,
            )
            # L^T[j, i] = exp(cs[i] - cs[j]) masked to j <= i
            ps_rbc = ps_tile([128, 128])
            nc.tensor.matmul(
                out=ps_rbc[:], lhsT=ones_row[:],
                rhs=cs_row0[:, col * 128 : (col + 1) * 128],
                start=True, stop=True,
            )
            rowm = lt_p.tile([128, 128], F32, tag="rowm", name=f"rowm{r}_{c}")
            nc.vector.tensor_tensor(
                out=rowm[:], in0=ps_rbc[:], in1=trimask[:], op=OP.add,
            )
            lt = lt_p.tile([128, 128], BF16, tag="lt", name=f"lt{r}_{c}")
            nc.scalar.activation(
                lt[:], rowm[:], AF.Exp, bias=negcs_col[:, col : col + 1]
            )
            ps_c = psc_p.tile([T, D], F32, tag="psc", name=f"psc{r}_{c}")
            if c == 0:
                nc.tensor.matmul(
                    out=ps_c[:], lhsT=lt[:], rhs=x_c[:], start=True, stop=True
                )
            else:
                nc.tensor.matmul(
                    out=ps_c[:], lhsT=lt[:], rhs=x_c[:], start=True, stop=False
                )
                nc.tensor.matmul(
                    out=ps_c[:], lhsT=carry_all[r][:, c * 128 : (c + 1) * 128],
                    rhs=smn_prev[:], start=False, stop=True,
                )
            smn = smn_p.tile([T, D], BF16, tag="smn", name=f"smn{r}_{c}")
            if c % 2 == 0:
                nc.vector.tensor_copy(smn[:], ps_c[:])
                nc.sync.dma_start(smoothed[r][c * T : (c + 1) * T, :], smn[:])
            else:
                nc.scalar.copy(smn[:], ps_c[:])
                nc.scalar.dma_start(smoothed[r][c * T : (c + 1) * T, :], smn[:])
            smn_prev = smn

    # --- indices, scatter-wrap: idx16[q, t] = clip(cumsum-1)[q*256 + t] ---
    idx_rep = []

    def idx_row(r):
        # W[q, t] = bd[t*16 + q] loaded directly (natural fed wrap)
        w4 = idx_p.tile([16, 256], F32, tag=f"w{r}")
        nc.gpsimd.dma_start(
            w4[:], bdry[r, :].rearrange("(t q) -> q t", q=16)
        )  # u8 -> f32 cast
        # column sums -> exclusive prefix along t
        pcs = ps_tile([1, 256])
        nc.tensor.matmul(
            out=pcs[:], lhsT=ones_col16[:], rhs=w4[:], start=True, stop=True
        )
        cs_sb = idx_p.tile([1, 256], F32, tag=f"cssb{r}")
        nc.vector.tensor_copy(cs_sb[:], pcs[:])
        incl = idx_p.tile([1, 256], F32, tag=f"incl{r}")
        nc.vector.tensor_tensor_scan(
            out=incl[:], data0=cs_sb[:], data1=zeros_row256[:],
            initial=0.0, op0=OP.add, op1=OP.add,
        )
        excl = idx_p.tile([1, 256], F32, tag=f"excl{r}")
        nc.vector.tensor_tensor(out=excl[:], in0=incl[:], in1=cs_sb[:], op=OP.subtract)
        # full cumsum = tri16 @ W + broadcast(excl)
        pidx = ps_tile([16, 256])
        nc.tensor.matmul(out=pidx[:], lhsT=tri16[:], rhs=w4[:], start=True, stop=False)
        nc.tensor.matmul(
            out=pidx[:], lhsT=ones_row[:, :16], rhs=excl[:], start=False, stop=True
        )
        idxf = idx_p.tile([16, 256], F32, tag=f"idxf{r}")
        nc.vector.tensor_scalar(
            out=idxf[:], in0=pidx[:], scalar1=-1.0, scalar2=0.0, op0=OP.add, op1=OP.max
        )
        nc.vector.tensor_scalar_min(idxf[:], idxf[:], float(J - 1))
        idx16 = idx_p.tile([16, 256], I16, tag=f"idx16{r}")
        nc.vector.tensor_copy(idx16[:], idxf[:])
        # replicate to 128 partitions (8 gpsimd core groups) by doubling
        rep = idx_p.tile([128, 256], I16, tag=f"rep{r}")
        nc.sync.dma_start(rep[0:16, :], idx16[:])
        nc.sync.dma_start(rep[16:32, :], rep[0:16, :])
        nc.sync.dma_start(rep[32:64, :], rep[0:32, :])
        nc.sync.dma_start(rep[64:128, :], rep[0:64, :])
        idx_rep.append(rep)

    def gather_sub(r, s):
        gt = gout_p.tile([128, GSUB, D], BF16, tag="gout", name=f"gout{r}_{s}")
        nc.gpsimd.dma_gather(
            out_ap=gt[:],
            in_ap=smoothed[r][:],
            idxs_ap=idx_rep[r][:, s * (SUBL // 16) : (s + 1) * (SUBL // 16)],
            num_idxs=SUBL,
            num_idxs_reg=SUBL,
            elem_size=D,
            queue_num=(r * NSUB + s) % 3 + 1,  # queue 0 is the mainline ring
        )
        return gt

    def store_sub(r, s, gt):
        # gt[p, g] holds frame l = 1024 s + 128 g + p (natural wrap)
        dst = out[r, s * SUBL : (s + 1) * SUBL, :].rearrange(
            "(g p) d -> p g d", p=128
        )
        if s % 2 == 0:
            nc.gpsimd.dma_start(dst, gt[:])  # bf16 -> f32 cast in DMA
        else:
            stg = stg_p.tile([128, GSUB, D], F32, tag="stg", name=f"stg{r}_{s}")
            if (r + s) % 4 < 2:
                nc.vector.tensor_copy(stg[:], gt[:])
            else:
                nc.scalar.copy(stg[:], gt[:])
            nc.sync.dma_start(dst, stg[:])

    ema_row(0)
    idx_row(0)
    gts0 = [gather_sub(0, s) for s in range(NSUB)]
    idx_row(1)
    ema_row(1)
    gts1 = []
    for s in range(NSUB):
        store_sub(0, s, gts0[s])
        gts1.append(gather_sub(1, s))
    for s in range(NSUB):
        store_sub(1, s, gts1[s])


def _patch_swdge_lane_by_queue():
    """Tile assigns DMASW completion-sem lanes round-robin, queue-blind; the
    HW/sim lock each lane's sem to one SWDGE queue. Pin lane = queue_num so
    multi-queue pool DMAs get consistent lanes."""
    from concourse import bass_isa
    from concourse import tile_sem_assignment as tsa

    if getattr(tsa.TileClockTick, "_ema_queue_patch", False):
        return
    orig = tsa.TileClockTick._assign_tick

    def patched(self, inst):
        if (
            isinstance(inst, bass_isa.AnyDMAInstruction)
            and inst.engine == mybir.EngineType.Pool
            and not isinstance(inst, bass_isa.UserSyncedRemoteDMADescs)
        ):
            self.next_sw_dma_idx = getattr(inst, "queue_num", 0) or 0
        return orig(self, inst)

    tsa.TileClockTick._assign_tick = patched
    tsa.TileClockTick._ema_queue_patch = True


def build():
    _patch_swdge_lane_by_queue()
    nc = bacc.Bacc(
        "TRN2",
        target_bir_lowering=False,
        debug=False,
        enable_asserts=False,
        num_devices=N_CORES,
        num_swdge_queues=4,
        dynamic_dma_scratch_size=16384,
    )
    with tile.TileContext(nc) as tc, ExitStack() as ctx:
        _body(tc, ctx)
    nc.compile()
    return nc


def make_in_maps(inputs):
    emb = np.asarray(inputs["unit_embeddings"], dtype=np.float32)
    conf = np.asarray(inputs["unit_confidence"], dtype=np.float32)
    msk = np.asarray(inputs["unit_mask"]).astype(np.uint8)
    bd = np.asarray(inputs["boundary_mask"]).astype(np.uint8)
    in_maps = []
    for c in range(N_CORES):
        sl = slice(c * BL, (c + 1) * BL)
        in_maps.append(
            {
                "unit_embeddings": np.ascontiguousarray(emb[sl]),
                "unit_confidence": np.ascontiguousarray(conf[sl]),
                "unit_mask": np.ascontiguousarray(msk[sl]),
                "boundary_mask": np.ascontiguousarray(bd[sl]),
            }
        )
    return in_maps


_cached_nc = None


def run(inputs, trace=False):
    global _cached_nc
    if _cached_nc is None:
        _cached_nc = build()
    res = run_bass_kernel_spmd(
        _cached_nc, make_in_maps(inputs), core_ids=list(range(N_CORES)), trace=trace
    )
    full = np.concatenate(
        [res.results[c]["frames"] for c in range(N_CORES)], axis=0
    )
    return full, res


def kernel(**inputs) -> np.ndarray:
    import os

    # Trace capture needs hooks absent outside our dev harness; make sure a
    # stray BASS_TRACE env can't route the grading run down that path.
    prev = os.environ.get("BASS_NEVER_TRACE")
    os.environ["BASS_NEVER_TRACE"] = "1"
    try:
        full, _ = run(inputs, trace=False)
    finally:
        if prev is None:
            os.environ.pop("BASS_NEVER_TRACE", None)
        else:
            os.environ["BASS_NEVER_TRACE"] = prev
    return full
